# revision 28
# baseline (speedup 1.0000x reference)
"""MoE top-2 dispatch -> per-expert Linear -> gated combine, on 8 TRN2 cores.

Single fused NEFF, transposed formulation.  Tokens are grouped by their
expert-pair "type" {e1,e2}; type {i,j} is split into one chunk on core i and
one on core j (star dispatch: every chunk on core c contains expert c), so a
core's groups all share slot-0 = its center expert.  Per group the device
computes out^T[o, tok] = g_a*(W_a^T x) + g_b*(W_b^T x) with tokens on the
matmul FREE dim (no 128-token tile quantization) and the top-2 combine done
for free by PSUM accumulation across the two experts.  Gates are folded into
x by a partition-broadcast (ones-matmul) + elementwise multiply on DVE.

Host side does dispatch bookkeeping only (gather/transpose/permute, zero
FLOPs).  Self-contained: shapes hardcoded for B=16384, E=8, D=1024, O=1024.
"""

import os
import sys
import types

sys.path.insert(0, "/opt/trn_rl_repo")

import ml_dtypes
import numpy as np

import concourse.bass as bass
import concourse.mybir as mybir
from concourse import bass_utils
from concourse.tile import TileContext

B, E, D, O = 16384, 8, 1024, 1024
N_CORES = 8
P = 128
KB = D // P   # contraction blocks (8)
NOB = O // P  # output row blocks (8)

_DT_MAP = {
    "float16": (mybir.dt.float16, np.float16),
    "bfloat16": (mybir.dt.bfloat16, ml_dtypes.bfloat16),
    "float32r": (mybir.dt.float32r, np.float32),
    "float32": (mybir.dt.float32, np.float32),
}

MAX_WAITS = int(os.environ.get("MOE_MAX_WAITS", "1"))


def _patch_tile_drain():
    """Public-walrus workaround: walrus codegen rejects instructions carrying
    more than a couple of sync-wait commands.  Tile's add_semaphores can put
    several waits on one instruction (and the kernel-tail drain carries one
    per live processor).  Hoist excess waits onto single-wait nop carriers
    emitted just before the instruction on the same engine."""
    from concourse.tile import TileContext as TC
    from concourse.vector_clock import ScopedClock

    if getattr(TC, "_moe_drain_patched", False):
        return

    orig_add = TC._add_instruction

    def _add_instruction(self, inst):
        si = getattr(inst, "sync_info", None)
        waits = list(si.on_wait or []) if si is not None else []
        if len(waits) > MAX_WAITS:
            hoist = waits[: len(waits) - MAX_WAITS]
            keep = waits[len(waits) - MAX_WAITS :]
            for w in hoist:
                nop = mybir.InstNoOp(
                    name=self.nc.get_next_instruction_name(),
                    engine=inst.engine,
                    bass_nofuse=True,
                    sync_info=mybir.SyncInfo(on_wait=[w], on_update=[]),
                )
                orig_add(self, nop)
            inst.sync_info = mybir.SyncInfo(
                on_wait=keep, on_update=list(si.on_update or [])
            )
        orig_add(self, inst)

    def _drain_and_barrier(self, tick_clock, wait_clock):
        carrier = self.nc.sync.nop(nofuse=True)
        wait_clock.add_sem_waits(
            carrier.ins, ScopedClock({None: tick_clock.global_clock})
        )
        si = carrier.ins.sync_info
        waits = list(si.on_wait or []) if si is not None else []
        if len(waits) > 1:
            carrier.ins.sync_info = mybir.SyncInfo(
                on_wait=waits[:1], on_update=list(si.on_update or [])
            )
            for w in waits[1:]:
                extra = self.nc.sync.nop(nofuse=True)
                extra.ins.sync_info = mybir.SyncInfo(on_wait=[w], on_update=[])
        self.nc.sync.drain()
        self.nc.all_engine_barrier()
        assert self.sems is not None
        popped = self.nc._tile_sem_poison_stack.pop()
        assert popped is self._sem_poison
        self.nc.clear_and_free_semaphores(list(self.sems.allocated().values()))
        self.nc.all_engine_barrier()

    TC._add_instruction = _add_instruction
    TC._drain_and_barrier = _drain_and_barrier
    TC._moe_drain_patched = True


_MASK = ~np.eye(E, dtype=bool)


def _profile(a):
    """Per-core chunk sizes sorted descending: [E, E-1]."""
    return -np.sort(-a[_MASK].reshape(E, E - 1), axis=1)


def _plan_splits(nmat):
    """Split each type {i,j} into chunks a[i,j] (on core i) and a[j,i]
    (on core j), minimizing CT = sum_k max_c (k-th largest chunk of core c),
    i.e. the canonical padded column count of the SPMD program.
    Simulated annealing over the 28 split points."""
    a0 = np.zeros((E, E), np.int64)
    for i in range(E):
        for j in range(i + 1, E):
            n = int(nmat[i, j])
            a0[i, j] = n // 2
            a0[j, i] = n - n // 2

    def ct_of(a):
        return int(_profile(a).max(0).sum())

    pairs = [(i, j) for i in range(E) for j in range(i + 1, E)]
    deltas = [1, -1, 2, -2, 4, -4, 8, -8, 16, -16, 32, -32, 64, -64]
    best_a, best_ct = a0.copy(), ct_of(a0)
    iters = int(os.environ.get("MOE_PLAN_ITERS", "150000"))
    for seed in range(2):
        rng = np.random.default_rng(seed)
        a = a0.copy()
        cur = float(ct_of(a))
        T0, T1 = 60.0, 0.05
        for t in range(iters):
            T = T0 * (T1 / T0) ** (t / iters)
            i, j = pairs[int(rng.integers(len(pairs)))]
            d = deltas[int(rng.integers(len(deltas)))]
            n = int(nmat[i, j])
            na = int(a[i, j]) + d
            if na < 0 or na > n:
                continue
            old = int(a[i, j])
            a[i, j] = na
            a[j, i] = n - na
            sc = float(ct_of(a))
            if sc <= cur or rng.random() < np.exp(-(sc - cur) / max(T, 1e-9)):
                cur = sc
                if sc < best_ct:
                    best_ct, best_a = int(sc), a.copy()
            else:
                a[i, j] = old
                a[j, i] = n - old
    return best_a, best_ct


def _route(gates):
    """Global dispatch plan.  Returns (plans, positions) where positions is
    the canonical group list [(slot, F)] (slot = partner W slot, 1-based;
    consecutive repeats share W) and plans[c] = (perm, tok_cols, g2, real):
      perm     : slot -> expert permutation (slot 0 = center = c)
      tok_cols : [CT] global token id per column (pads = 0)
      g2       : [2, CT] gate for slot-a (center) / slot-b (partner)
      real     : [CT] bool, True where the column is a real token
    """
    g = np.asarray(gates)
    order = np.argsort(-g, axis=1)[:, :2]
    e_lo = np.minimum(order[:, 0], order[:, 1])
    e_hi = np.maximum(order[:, 0], order[:, 1])
    nmat = np.zeros((E, E), np.int64)
    np.add.at(nmat, (e_lo, e_hi), 1)
    nmat = nmat + nmat.T

    a, _ = _plan_splits(nmat)

    # token lists per type; first a[i,j] tokens of {i,j} -> core i, rest -> j
    chunk_toks = {}
    for i in range(E):
        for j in range(i + 1, E):
            toks = np.nonzero((e_lo == i) & (e_hi == j))[0]
            ai = int(a[i, j])
            chunk_toks[(i, j)] = toks[:ai]
            chunk_toks[(j, i)] = toks[ai:]

    # canonical rank sizes: need[k] = max over cores of k-th largest chunk;
    # ranks > 512 split into equal sub-positions (PSUM bank = 512 fp32 cols)
    need = _profile(a).max(0)
    positions = []  # (rank k, slot k+1, F)
    for k in range(E - 1):
        n = int(need[k])
        if n <= 0:
            continue
        m = -(-n // 512)
        base, rem = divmod(n, m)
        for s in range(m):
            positions.append((k, k + 1, base + (1 if s < rem else 0)))
    CT = sum(f for _k, _s, f in positions)

    plans = []
    for c in range(E):
        partners = [p for p in range(E) if p != c]
        partners.sort(key=lambda p: -len(chunk_toks[(c, p)]))
        perm = [c] + partners
        tok_cols = np.zeros(CT, np.int64)
        g2 = np.zeros((2, CT), np.float32)
        real = np.zeros(CT, bool)
        cursor = [0] * (E - 1)
        off = 0
        for k, _slot, f in positions:
            p = partners[k]
            toks = chunk_toks[(c, p)][cursor[k] : cursor[k] + f]
            cursor[k] += len(toks)
            n = len(toks)
            tok_cols[off : off + n] = toks
            g2[0, off : off + n] = g[toks, c]
            g2[1, off : off + n] = g[toks, p]
            real[off : off + n] = True
            off += f
        assert all(
            cursor[k] == len(chunk_toks[(c, partners[k])]) for k in range(E - 1)
        ), "unplaced tokens"
        plans.append((perm, tok_cols, g2, real))
    return plans, [(s, f) for _k, s, f in positions]


def _build_core_inputs(x, W, b, plan, positions, np_dt, bias_flag):
    perm, tok_cols, g2, _real = plan
    CT = len(tok_cols)
    F = [f for _s, f in positions]
    offs = np.concatenate([[0], np.cumsum(F)])
    xt3 = (
        x[tok_cols]
        .astype(np_dt)
        .reshape(CT, KB, P)
        .transpose(2, 1, 0)
    )  # [128(ki), KB, CT]
    # flat per-group layout: [128, sum_k KB*F_k], each group contiguous
    # per partition so its DMA is a single run per partition
    xt = np.concatenate(
        [
            xt3[:, :, offs[k] : offs[k + 1]].reshape(P, KB * F[k])
            for k in range(len(F))
        ],
        axis=1,
    ).copy()
    w = (
        W[perm]
        .astype(np_dt)
        .reshape(E, KB, P, O)
        .transpose(0, 2, 1, 3)
        .copy()
    )  # [slot, 128(ki), KB, O]
    m = {
        "xt": xt,
        "w": w,
        "g2": g2.astype(np_dt),
        # gate rows replicated across partitions: DMA'd straight into the
        # [128, f] per-group gate tiles (host-side bookkeeping, no FLOPs)
        "grep": np.ascontiguousarray(
            np.broadcast_to(g2.astype(np_dt)[:, None, :], (2, P, CT))
        ),
    }
    if bias_flag:
        G = len(positions)
        b2 = np.zeros((2, G, O), np.float32)
        b2[0, :, :] = b[perm[0]]
        for k, (slot, _f) in enumerate(positions):
            b2[1, k, :] = b[perm[slot]]
        m["b2"] = b2.astype(np_dt)
    return m


def _build_program(positions, dt, bias_flag):
    """One fused NEFF: per group k (columns c0:c0+F[k]) accumulate in PSUM
    out^T[o_block] = W_slot0^T (x*g_a) + W_slotk^T (x*g_b) (+ bias via a
    rank-2 matmul with the gate rows), evict through the scalar engine."""
    G = len(positions)
    slots = [s for s, _f in positions]
    F = [f for _s, f in positions]
    CT = sum(F)
    KH = KB // 2  # W dma chunk: half the contraction blocks (contiguous)
    nc = bass.Bass(target_bir_lowering=False, trn_type="TRN2")
    xt_d = nc.dram_tensor("xt", [P, KB * CT], dt, kind="ExternalInput")
    w_d = nc.dram_tensor("w", [E, P, KB, O], dt, kind="ExternalInput")
    g_d = nc.dram_tensor("g2", [2, CT], dt, kind="ExternalInput")
    grep_d = nc.dram_tensor("grep", [2, P, CT], dt, kind="ExternalInput")
    if bias_flag:
        b_d = nc.dram_tensor("b2", [2, G, O], dt, kind="ExternalInput")
    out_d = nc.dram_tensor("out", [P, NOB * CT], dt, kind="ExternalOutput")

    offs = np.concatenate([[0], np.cumsum(F)])

    with TileContext(nc) as tc:
        with (
            tc.tile_pool(name="const", bufs=1) as cpool,
            tc.tile_pool(name="wp", bufs=3) as wpool,
            tc.tile_pool(name="xtp", bufs=3) as xtpool,
            tc.tile_pool(name="xg", bufs=32) as xgpool,
            tc.tile_pool(name="gs", bufs=4) as gspool,
            tc.tile_pool(name="ot", bufs=4) as opool,
            tc.tile_pool(name="ps", bufs=6, space="PSUM") as pspool,
        ):
            if bias_flag:
                g_sb = cpool.tile([2, CT], dt)
                nc.sync.dma_start(out=g_sb[:], in_=g_d[:, :])
                b_sb = cpool.tile([2, G, O], dt)
                nc.sync.dma_start(out=b_sb[:], in_=b_d[:, :, :])
            # center expert weights, resident; two contiguous KB-half chunks
            # so only the first chunk gates the first matmul
            w0 = [None, None]

            def load_w0(h):
                w0t = cpool.tile([P, KH, O], dt, name=f"w0_{h}")
                nc.scalar.dma_start(
                    out=w0t[:], in_=w_d[0, :, h * KH : (h + 1) * KH, :]
                )
                w0[h] = w0t

            xt_t = [None] * G
            w_t = [None] * G
            xg_t = [None] * G

            def prepare_w(k, halves):
                if k > 0 and slots[k] == slots[k - 1]:
                    w_t[k] = w_t[k - 1]  # sub-position: same partner W
                    return
                if w_t[k] is None:
                    w_t[k] = [None, None]
                for h in halves:
                    wt = wpool.tile([P, KH, O], dt, tag=f"w{h}")
                    nc.sync.dma_start(
                        out=wt[:],
                        in_=w_d[slots[k], :, h * KH : (h + 1) * KH, :],
                    )
                    w_t[k][h] = wt

            def prepare_xg(k):
                c0, f = int(offs[k]), F[k]
                xt = xtpool.tile([P, KB * f], dt, tag="xt")
                nc.scalar.dma_start(
                    out=xt[:], in_=xt_d[:, KB * c0 : KB * c0 + KB * f]
                )
                xt_t[k] = xt
                xgs = []
                for s in range(2):
                    gs = gspool.tile([P, f], dt, tag="Gs")
                    nc.sync.dma_start(
                        out=gs[:], in_=grep_d[s, :, c0 : c0 + f]
                    )
                    row = []
                    for kb in range(KB):
                        xg = xgpool.tile([P, f], dt, tag="xg")
                        nc.vector.tensor_mul(
                            out=xg[:],
                            in0=xt[:, kb * f : (kb + 1) * f],
                            in1=gs[:],
                        )
                        row.append(xg)
                    xgs.append(row)
                xg_t[k] = xgs

            def compute(k):
                c0, f = int(offs[k]), F[k]
                xgs = xg_t[k]
                o_t = opool.tile([P, NOB * f], dt, tag="o")
                for ob in range(NOB):
                    ps = pspool.tile([P, f], mybir.dt.float32, tag="ps")
                    first = True
                    for h in range(2):
                        for s in range(2):
                            wsrc = w0[h] if s == 0 else w_t[k][h]
                            for kb in range(KH):
                                nc.tensor.matmul(
                                    out=ps[:],
                                    lhsT=wsrc[:, kb, ob * P : (ob + 1) * P],
                                    rhs=xgs[s][h * KH + kb][:],
                                    start=first,
                                    stop=(
                                        h == 1
                                        and s == 1
                                        and kb == KH - 1
                                        and not bias_flag
                                    ),
                                )
                                first = False
                    if bias_flag:
                        nc.tensor.matmul(
                            out=ps[:],
                            lhsT=b_sb[0:2, k, ob * P : (ob + 1) * P],
                            rhs=g_sb[0:2, c0 : c0 + f],
                            start=False,
                            stop=True,
                        )
                    # alternate eviction engine: scalar / vector
                    if ob % 2 == 0:
                        nc.scalar.copy(
                            out=o_t[:, ob * f : (ob + 1) * f], in_=ps[:]
                        )
                    else:
                        nc.vector.tensor_copy(
                            out=o_t[:, ob * f : (ob + 1) * f], in_=ps[:]
                        )
                nc.sync.dma_start(
                    out=out_d[:, NOB * c0 : NOB * c0 + NOB * f], in_=o_t[:]
                )
                # release references so pools can recycle
                xg_t[k] = None
                w_t[k] = None
                xt_t[k] = None

            # startup: first xt + first W chunks gate the first matmuls;
            # stream the rest behind them
            prepare_w(0, [0])
            load_w0(0)
            prepare_xg(0)
            prepare_w(0, [1])
            load_w0(1)
            if G > 1:
                prepare_w(1, range(2))
                prepare_xg(1)
            for k in range(G):
                compute(k)
                if k + 2 < G:
                    prepare_w(k + 2, range(2))
                    prepare_xg(k + 2)
    return nc


def kernel(x, gates, W, b):
    _patch_tile_drain()
    dt_name = os.environ.get("MOE_DT", "float16")
    dt, np_dt = _DT_MAP[dt_name]
    bias_flag = bool(np.any(b != 0))

    gates = np.asarray(gates)
    x = np.ascontiguousarray(x)
    W = np.asarray(W)
    b = np.asarray(b)

    plans, positions = _route(gates)
    in_maps = [
        _build_core_inputs(x, W, b, plans[c], positions, np_dt, bias_flag)
        for c in range(N_CORES)
    ]
    nc = _build_program(positions, dt, bias_flag)

    trace = os.environ.get("MOE_TRACE", "0") == "1"
    kwargs = {}
    if trace:
        _install_ntff_shim()
        kwargs = dict(trace=True, trace_cores=list(range(N_CORES)))

    res = bass_utils.run_bass_kernel_spmd(
        nc, in_maps, core_ids=list(range(N_CORES)), **kwargs
    )
    if trace and res.exec_time_ns is not None:
        print(
            f"HW exec time: {res.exec_time_ns} ns "
            f"(mean {res.mean_exec_time_ns:.0f})"
        )
    F = [f for _s, f in positions]
    offs = np.concatenate([[0], np.cumsum(F)])
    out = np.empty((B, O), np.float32)
    for c in range(N_CORES):
        perm, tok_cols, _g2, real = plans[c]
        CT = len(tok_cols)
        arr = res.results[c]["out"].reshape(P, NOB * CT)
        for k, f in enumerate(F):
            c0 = int(offs[k])
            blk = (
                arr[:, NOB * c0 : NOB * c0 + NOB * f]
                .reshape(P, NOB, f)
                .transpose(1, 0, 2)
                .reshape(O, f)
            )
            r = real[c0 : c0 + f]
            toks = tok_cols[c0 : c0 + f][r]
            out[toks] = blk[:, r].T.astype(np.float32)
    return out


def _install_ntff_shim():
    """Best-effort: register the missing antenv.axon_hooks NTFF profile hook
    so trace=True yields exec_time_ns.  Only used when MOE_TRACE=1."""
    try:
        import antenv
        from trn_agent_boot.trn_boot import _ntff_profile_via_ctypes

        if "antenv.axon_hooks" in sys.modules:
            return
        hooks = types.ModuleType("antenv.axon_hooks")
        hook = _ntff_profile_via_ctypes("/opt/axon/libaxon_pjrt.so")
        hooks.get_axon_ntff_profile_hook = lambda: hook
        hooks.set_axon_ntff_profile_hook = lambda h: None
        sys.modules["antenv.axon_hooks"] = hooks
        antenv.axon_hooks = hooks
        bass_utils.upload_artifacts = lambda tmpdir: tmpdir
    except Exception as e:  # pragma: no cover
        print(f"ntff shim unavailable: {e}", file=sys.stderr)


# revision 31
# speedup vs baseline: 1.0173x; 1.0173x over previous
"""MoE top-2 dispatch -> per-expert Linear -> gated combine, on 8 TRN2 cores.

Single fused NEFF, transposed formulation.  Tokens are grouped by their
expert-pair "type" {e1,e2}; type {i,j} is split into one chunk on core i and
one on core j (star dispatch: every chunk on core c contains expert c), so a
core's groups all share slot-0 = its center expert.  Per group the device
computes out^T[o, tok] = g_a*(W_a^T x) + g_b*(W_b^T x) with tokens on the
matmul FREE dim (no 128-token tile quantization) and the top-2 combine done
for free by PSUM accumulation across the two experts.  Gates are folded into
x by a partition-broadcast (ones-matmul) + elementwise multiply on DVE.

Host side does dispatch bookkeeping only (gather/transpose/permute, zero
FLOPs).  Self-contained: shapes hardcoded for B=16384, E=8, D=1024, O=1024.
"""

import os
import sys
import types

sys.path.insert(0, "/opt/trn_rl_repo")

import ml_dtypes
import numpy as np

import concourse.bass as bass
import concourse.mybir as mybir
from concourse import bass_utils
from concourse.tile import TileContext

B, E, D, O = 16384, 8, 1024, 1024
N_CORES = 8
P = 128
KB = D // P   # contraction blocks (8)
NOB = O // P  # output row blocks (8)

_DT_MAP = {
    "float16": (mybir.dt.float16, np.float16),
    "bfloat16": (mybir.dt.bfloat16, ml_dtypes.bfloat16),
    "float32r": (mybir.dt.float32r, np.float32),
    "float32": (mybir.dt.float32, np.float32),
}

MAX_WAITS = int(os.environ.get("MOE_MAX_WAITS", "1"))


def _patch_tile_drain():
    """Public-walrus workaround: walrus codegen rejects instructions carrying
    more than a couple of sync-wait commands.  Tile's add_semaphores can put
    several waits on one instruction (and the kernel-tail drain carries one
    per live processor).  Hoist excess waits onto single-wait nop carriers
    emitted just before the instruction on the same engine."""
    from concourse.tile import TileContext as TC
    from concourse.vector_clock import ScopedClock

    if getattr(TC, "_moe_drain_patched", False):
        return

    orig_add = TC._add_instruction

    def _add_instruction(self, inst):
        si = getattr(inst, "sync_info", None)
        waits = list(si.on_wait or []) if si is not None else []
        if len(waits) > MAX_WAITS:
            hoist = waits[: len(waits) - MAX_WAITS]
            keep = waits[len(waits) - MAX_WAITS :]
            for w in hoist:
                nop = mybir.InstNoOp(
                    name=self.nc.get_next_instruction_name(),
                    engine=inst.engine,
                    bass_nofuse=True,
                    sync_info=mybir.SyncInfo(on_wait=[w], on_update=[]),
                )
                orig_add(self, nop)
            inst.sync_info = mybir.SyncInfo(
                on_wait=keep, on_update=list(si.on_update or [])
            )
        orig_add(self, inst)

    def _drain_and_barrier(self, tick_clock, wait_clock):
        carrier = self.nc.sync.nop(nofuse=True)
        wait_clock.add_sem_waits(
            carrier.ins, ScopedClock({None: tick_clock.global_clock})
        )
        si = carrier.ins.sync_info
        waits = list(si.on_wait or []) if si is not None else []
        if len(waits) > 1:
            carrier.ins.sync_info = mybir.SyncInfo(
                on_wait=waits[:1], on_update=list(si.on_update or [])
            )
            for w in waits[1:]:
                extra = self.nc.sync.nop(nofuse=True)
                extra.ins.sync_info = mybir.SyncInfo(on_wait=[w], on_update=[])
        self.nc.sync.drain()
        self.nc.all_engine_barrier()
        assert self.sems is not None
        popped = self.nc._tile_sem_poison_stack.pop()
        assert popped is self._sem_poison
        self.nc.clear_and_free_semaphores(list(self.sems.allocated().values()))
        self.nc.all_engine_barrier()

    TC._add_instruction = _add_instruction
    TC._drain_and_barrier = _drain_and_barrier
    TC._moe_drain_patched = True


_MASK = ~np.eye(E, dtype=bool)


def _profile(a):
    """Per-core chunk sizes sorted descending: [E, E-1]."""
    return -np.sort(-a[_MASK].reshape(E, E - 1), axis=1)


def _plan_splits(nmat):
    """Split each type {i,j} into chunks a[i,j] (on core i) and a[j,i]
    (on core j), minimizing CT = sum_k max_c (k-th largest chunk of core c),
    i.e. the canonical padded column count of the SPMD program.
    Simulated annealing over the 28 split points."""
    a0 = np.zeros((E, E), np.int64)
    for i in range(E):
        for j in range(i + 1, E):
            n = int(nmat[i, j])
            a0[i, j] = n // 2
            a0[j, i] = n - n // 2

    def ct_of(a):
        return int(_profile(a).max(0).sum())

    pairs = [(i, j) for i in range(E) for j in range(i + 1, E)]
    deltas = [1, -1, 2, -2, 4, -4, 8, -8, 16, -16, 32, -32, 64, -64]
    best_a, best_ct = a0.copy(), ct_of(a0)
    iters = int(os.environ.get("MOE_PLAN_ITERS", "150000"))
    for seed in range(2):
        rng = np.random.default_rng(seed)
        a = a0.copy()
        cur = float(ct_of(a))
        T0, T1 = 60.0, 0.05
        for t in range(iters):
            T = T0 * (T1 / T0) ** (t / iters)
            i, j = pairs[int(rng.integers(len(pairs)))]
            d = deltas[int(rng.integers(len(deltas)))]
            n = int(nmat[i, j])
            na = int(a[i, j]) + d
            if na < 0 or na > n:
                continue
            old = int(a[i, j])
            a[i, j] = na
            a[j, i] = n - na
            sc = float(ct_of(a))
            if sc <= cur or rng.random() < np.exp(-(sc - cur) / max(T, 1e-9)):
                cur = sc
                if sc < best_ct:
                    best_ct, best_a = int(sc), a.copy()
            else:
                a[i, j] = old
                a[j, i] = n - old
    return best_a, best_ct


def _route(gates):
    """Global dispatch plan.  Returns (plans, positions) where positions is
    the canonical group list [(slot, F)] (slot = partner W slot, 1-based;
    consecutive repeats share W) and plans[c] = (perm, tok_cols, g2, real):
      perm     : slot -> expert permutation (slot 0 = center = c)
      tok_cols : [CT] global token id per column (pads = 0)
      g2       : [2, CT] gate for slot-a (center) / slot-b (partner)
      real     : [CT] bool, True where the column is a real token
    """
    g = np.asarray(gates)
    order = np.argsort(-g, axis=1)[:, :2]
    e_lo = np.minimum(order[:, 0], order[:, 1])
    e_hi = np.maximum(order[:, 0], order[:, 1])
    nmat = np.zeros((E, E), np.int64)
    np.add.at(nmat, (e_lo, e_hi), 1)
    nmat = nmat + nmat.T

    a, _ = _plan_splits(nmat)

    # token lists per type; first a[i,j] tokens of {i,j} -> core i, rest -> j
    chunk_toks = {}
    for i in range(E):
        for j in range(i + 1, E):
            toks = np.nonzero((e_lo == i) & (e_hi == j))[0]
            ai = int(a[i, j])
            chunk_toks[(i, j)] = toks[:ai]
            chunk_toks[(j, i)] = toks[ai:]

    # canonical rank sizes: need[k] = max over cores of k-th largest chunk;
    # ranks > 512 split into equal sub-positions (PSUM bank = 512 fp32 cols)
    need = _profile(a).max(0)
    positions = []  # (rank k, slot k+1, F)
    for k in range(E - 1):
        n = int(need[k])
        if n <= 0:
            continue
        m = -(-n // 512)
        base, rem = divmod(n, m)
        for s in range(m):
            positions.append((k, k + 1, base + (1 if s < rem else 0)))
    CT = sum(f for _k, _s, f in positions)

    plans = []
    for c in range(E):
        partners = [p for p in range(E) if p != c]
        partners.sort(key=lambda p: -len(chunk_toks[(c, p)]))
        perm = [c] + partners
        tok_cols = np.zeros(CT, np.int64)
        g2 = np.zeros((2, CT), np.float32)
        real = np.zeros(CT, bool)
        cursor = [0] * (E - 1)
        off = 0
        for k, _slot, f in positions:
            p = partners[k]
            toks = chunk_toks[(c, p)][cursor[k] : cursor[k] + f]
            cursor[k] += len(toks)
            n = len(toks)
            tok_cols[off : off + n] = toks
            g2[0, off : off + n] = g[toks, c]
            g2[1, off : off + n] = g[toks, p]
            real[off : off + n] = True
            off += f
        assert all(
            cursor[k] == len(chunk_toks[(c, partners[k])]) for k in range(E - 1)
        ), "unplaced tokens"
        plans.append((perm, tok_cols, g2, real))
    return plans, [(s, f) for _k, s, f in positions]


def _build_core_inputs(x, W, b, plan, positions, np_dt, bias_flag):
    perm, tok_cols, g2, _real = plan
    CT = len(tok_cols)
    F = [f for _s, f in positions]
    offs = np.concatenate([[0], np.cumsum(F)])
    xt3 = (
        x[tok_cols]
        .astype(np_dt)
        .reshape(CT, KB, P)
        .transpose(2, 1, 0)
    )  # [128(ki), KB, CT]
    # flat per-group layout: [128, sum_k KB*F_k], each group contiguous
    # per partition so its DMA is a single run per partition
    xt = np.concatenate(
        [
            xt3[:, :, offs[k] : offs[k + 1]].reshape(P, KB * F[k])
            for k in range(len(F))
        ],
        axis=1,
    ).copy()
    w = (
        W[perm]
        .astype(np_dt)
        .reshape(E, KB, P, O)
        .transpose(0, 2, 1, 3)
        .copy()
    )  # [slot, 128(ki), KB, O]
    m = {
        "xt": xt,
        "w": w,
        "g2": g2.astype(np_dt),
        # gate rows replicated across partitions: DMA'd straight into the
        # [128, f] per-group gate tiles (host-side bookkeeping, no FLOPs)
        "grep": np.ascontiguousarray(
            np.broadcast_to(g2.astype(np_dt)[:, None, :], (2, P, CT))
        ),
    }
    if bias_flag:
        G = len(positions)
        b2 = np.zeros((2, G, O), np.float32)
        b2[0, :, :] = b[perm[0]]
        for k, (slot, _f) in enumerate(positions):
            b2[1, k, :] = b[perm[slot]]
        m["b2"] = b2.astype(np_dt)
    return m


def _build_program(positions, dt, bias_flag):
    """One fused NEFF: per group k (columns c0:c0+F[k]) accumulate in PSUM
    out^T[o_block] = W_slot0^T (x*g_a) + W_slotk^T (x*g_b) (+ bias via a
    rank-2 matmul with the gate rows), evict through the scalar engine."""
    G = len(positions)
    slots = [s for s, _f in positions]
    F = [f for _s, f in positions]
    CT = sum(F)
    KH = KB // 2  # W dma chunk: half the contraction blocks (contiguous)
    nc = bass.Bass(target_bir_lowering=False, trn_type="TRN2")
    xt_d = nc.dram_tensor("xt", [P, KB * CT], dt, kind="ExternalInput")
    w_d = nc.dram_tensor("w", [E, P, KB, O], dt, kind="ExternalInput")
    g_d = nc.dram_tensor("g2", [2, CT], dt, kind="ExternalInput")
    grep_d = nc.dram_tensor("grep", [2, P, CT], dt, kind="ExternalInput")
    if bias_flag:
        b_d = nc.dram_tensor("b2", [2, G, O], dt, kind="ExternalInput")
    out_d = nc.dram_tensor("out", [P, NOB * CT], dt, kind="ExternalOutput")

    offs = np.concatenate([[0], np.cumsum(F)])

    with TileContext(nc) as tc:
        with (
            tc.tile_pool(name="const", bufs=1) as cpool,
            tc.tile_pool(name="wp", bufs=3) as wpool,
            tc.tile_pool(name="xtp", bufs=3) as xtpool,
            tc.tile_pool(name="xg", bufs=32) as xgpool,
            tc.tile_pool(name="gs", bufs=4) as gspool,
            tc.tile_pool(name="ot", bufs=4) as opool,
            tc.tile_pool(name="ps", bufs=6, space="PSUM") as pspool,
        ):
            if bias_flag:
                g_sb = cpool.tile([2, CT], dt)
                nc.sync.dma_start(out=g_sb[:], in_=g_d[:, :])
                b_sb = cpool.tile([2, G, O], dt)
                nc.sync.dma_start(out=b_sb[:], in_=b_d[:, :, :])
            # center expert weights, resident; two contiguous KB-half chunks
            # so only the first chunk gates the first matmul
            w0 = [None, None]

            def load_w0(h):
                w0t = cpool.tile([P, KH, O], dt, name=f"w0_{h}")
                nc.scalar.dma_start(
                    out=w0t[:], in_=w_d[0, :, h * KH : (h + 1) * KH, :]
                )
                w0[h] = w0t

            xt_t = [None] * G
            w_t = [None] * G
            xg_t = [None] * G

            def prepare_w(k, halves, eng=None):
                if k > 0 and slots[k] == slots[k - 1]:
                    w_t[k] = w_t[k - 1]  # sub-position: same partner W
                    return
                if w_t[k] is None:
                    w_t[k] = [None, None]
                for h in halves:
                    wt = wpool.tile([P, KH, O], dt, tag=f"w{h}")
                    (eng or nc.sync).dma_start(
                        out=wt[:],
                        in_=w_d[slots[k], :, h * KH : (h + 1) * KH, :],
                    )
                    w_t[k][h] = wt

            def prepare_xg(k):
                c0, f = int(offs[k]), F[k]
                xt = xtpool.tile([P, KB * f], dt, tag="xt")
                half = KH * f
                nc.scalar.dma_start(
                    out=xt[:, :half], in_=xt_d[:, KB * c0 : KB * c0 + half]
                )
                nc.scalar.dma_start(
                    out=xt[:, half:],
                    in_=xt_d[:, KB * c0 + half : KB * (c0 + f)],
                )
                xt_t[k] = xt
                xgs = []
                for s in range(2):
                    gs = gspool.tile([P, f], dt, tag="Gs")
                    nc.scalar.dma_start(
                        out=gs[:], in_=grep_d[s, :, c0 : c0 + f]
                    )
                    row = []
                    for kb in range(KB):
                        xg = xgpool.tile([P, f], dt, tag="xg")
                        nc.vector.tensor_mul(
                            out=xg[:],
                            in0=xt[:, kb * f : (kb + 1) * f],
                            in1=gs[:],
                        )
                        row.append(xg)
                    xgs.append(row)
                xg_t[k] = xgs

            def compute(k, mid=None):
                c0, f = int(offs[k]), F[k]
                xgs = xg_t[k]
                o_t = opool.tile([P, NOB * f], dt, tag="o")
                for ob in range(NOB):
                    ps = pspool.tile([P, f], mybir.dt.float32, tag="ps")
                    first = True
                    for h in range(2):
                        for s in range(2):
                            wsrc = w0[h] if s == 0 else w_t[k][h]
                            for kb in range(KH):
                                nc.tensor.matmul(
                                    out=ps[:],
                                    lhsT=wsrc[:, kb, ob * P : (ob + 1) * P],
                                    rhs=xgs[s][h * KH + kb][:],
                                    start=first,
                                    stop=(
                                        h == 1
                                        and s == 1
                                        and kb == KH - 1
                                        and not bias_flag
                                    ),
                                )
                                first = False
                    if bias_flag:
                        nc.tensor.matmul(
                            out=ps[:],
                            lhsT=b_sb[0:2, k, ob * P : (ob + 1) * P],
                            rhs=g_sb[0:2, c0 : c0 + f],
                            start=False,
                            stop=True,
                        )
                    # alternate eviction engine: scalar / vector
                    if ob % 2 == 0:
                        nc.scalar.copy(
                            out=o_t[:, ob * f : (ob + 1) * f], in_=ps[:]
                        )
                    else:
                        nc.vector.tensor_copy(
                            out=o_t[:, ob * f : (ob + 1) * f], in_=ps[:]
                        )
                    if ob == 0 and mid is not None:
                        # issue the next group's DMAs here: on the scalar
                        # stream they sit behind this group's first
                        # eviction, so the prefetch cannot flood the DMA
                        # rings before this group's own data has landed
                        mid()
                nc.sync.dma_start(
                    out=out_d[:, NOB * c0 : NOB * c0 + NOB * f], in_=o_t[:]
                )
                # release references so pools can recycle
                xg_t[k] = None
                w_t[k] = None
                xt_t[k] = None

            # startup: only group 0's data up front, in consumption order
            prepare_w(0, [0], eng=nc.sync)
            load_w0(0)
            prepare_xg(0)
            prepare_w(0, [1], eng=nc.sync)
            load_w0(1)

            def make_mid(k):
                if k >= G:
                    return None

                def mid():
                    prepare_w(k, range(2), eng=nc.scalar)
                    prepare_xg(k)

                return mid

            for k in range(G):
                compute(k, mid=make_mid(k + 1))
    return nc


def kernel(x, gates, W, b):
    _patch_tile_drain()
    dt_name = os.environ.get("MOE_DT", "float16")
    dt, np_dt = _DT_MAP[dt_name]
    bias_flag = bool(np.any(b != 0))

    gates = np.asarray(gates)
    x = np.ascontiguousarray(x)
    W = np.asarray(W)
    b = np.asarray(b)

    plans, positions = _route(gates)
    in_maps = [
        _build_core_inputs(x, W, b, plans[c], positions, np_dt, bias_flag)
        for c in range(N_CORES)
    ]
    nc = _build_program(positions, dt, bias_flag)

    trace = os.environ.get("MOE_TRACE", "0") == "1"
    kwargs = {}
    if trace:
        _install_ntff_shim()
        kwargs = dict(trace=True, trace_cores=list(range(N_CORES)))

    res = bass_utils.run_bass_kernel_spmd(
        nc, in_maps, core_ids=list(range(N_CORES)), **kwargs
    )
    if trace and res.exec_time_ns is not None:
        print(
            f"HW exec time: {res.exec_time_ns} ns "
            f"(mean {res.mean_exec_time_ns:.0f})"
        )
    F = [f for _s, f in positions]
    offs = np.concatenate([[0], np.cumsum(F)])
    out = np.empty((B, O), np.float32)
    for c in range(N_CORES):
        perm, tok_cols, _g2, real = plans[c]
        CT = len(tok_cols)
        arr = res.results[c]["out"].reshape(P, NOB * CT)
        for k, f in enumerate(F):
            c0 = int(offs[k])
            blk = (
                arr[:, NOB * c0 : NOB * c0 + NOB * f]
                .reshape(P, NOB, f)
                .transpose(1, 0, 2)
                .reshape(O, f)
            )
            r = real[c0 : c0 + f]
            toks = tok_cols[c0 : c0 + f][r]
            out[toks] = blk[:, r].T.astype(np.float32)
    return out


def _install_ntff_shim():
    """Best-effort: register the missing antenv.axon_hooks NTFF profile hook
    so trace=True yields exec_time_ns.  Only used when MOE_TRACE=1."""
    try:
        import antenv
        from trn_agent_boot.trn_boot import _ntff_profile_via_ctypes

        if "antenv.axon_hooks" in sys.modules:
            return
        hooks = types.ModuleType("antenv.axon_hooks")
        hook = _ntff_profile_via_ctypes("/opt/axon/libaxon_pjrt.so")
        hooks.get_axon_ntff_profile_hook = lambda: hook
        hooks.set_axon_ntff_profile_hook = lambda h: None
        sys.modules["antenv.axon_hooks"] = hooks
        antenv.axon_hooks = hooks
        bass_utils.upload_artifacts = lambda tmpdir: tmpdir
    except Exception as e:  # pragma: no cover
        print(f"ntff shim unavailable: {e}", file=sys.stderr)
